# revision 1
# baseline (speedup 1.0000x reference)
import sys
sys.path.insert(0, '/opt/trn_rl_repo')
import numpy as np

N, MA, F, NF = 512, 64, 128, 16
NCORES = 8
MPC = N // NCORES      # 64 molecules per core
NB = MPC // 2          # 32 batches of 2 molecules
DIN = F * (1 + NF)     # 2176
H1 = 512
UPDATE = 0.1 * 1.0     # UPDATE_RATIO * DECAY
WSCALE = 5.0

_NC = None


def _build():
    from concourse import bacc, tile, mybir
    dt = mybir.dt
    AF = mybir.ActivationFunctionType
    ALU = mybir.AluOpType

    nc = bacc.Bacc(trn_type="TRN2")
    HPACK = nc.dram_tensor("HPACK", [NB * 128, 132], dt.float32, kind="ExternalInput")
    AUGT = nc.dram_tensor("AUGT", [NB * 5, 256], dt.float32, kind="ExternalInput")
    SMU = nc.dram_tensor("SMU", [128, 2048], dt.float32, kind="ExternalInput")
    IDN = nc.dram_tensor("IDN", [128, 128], dt.float32, kind="ExternalInput")
    W1D = nc.dram_tensor("W1D", [DIN, H1], dt.float32, kind="ExternalInput")
    W2D = nc.dram_tensor("W2D", [H1, F], dt.float32, kind="ExternalInput")
    B1D = nc.dram_tensor("B1D", [1, H1], dt.float32, kind="ExternalInput")
    B2D = nc.dram_tensor("B2D", [1, F], dt.float32, kind="ExternalInput")
    ONED = nc.dram_tensor("ONED", [1, 128], dt.float32, kind="ExternalInput")
    HNEW = nc.dram_tensor("HNEW", [NB * 128, F], dt.float32, kind="ExternalOutput")

    with tile.TileContext(nc) as tc:
        with tc.tile_pool(name="wp", bufs=1) as wp, \
             tc.tile_pool(name="sb", bufs=2) as pool, \
             tc.tile_pool(name="ps", bufs=1, space="PSUM") as psum, \
             tc.tile_pool(name="pa", bufs=1, space="PSUM") as psa:
            # ---- one-time: weights / constants
            smu = wp.tile([128, 2048], dt.float32)
            ident = wp.tile([128, 128], dt.float32)
            w1f = wp.tile([128, 17 * H1], dt.float32)
            w2f = wp.tile([128, 4 * F], dt.float32)
            b1f = wp.tile([1, H1], dt.float32)
            b2f = wp.tile([1, F], dt.float32)
            onef = wp.tile([1, 128], dt.float32)
            nc.sync.dma_start(smu[:], SMU[:])
            nc.sync.dma_start(ident[:], IDN[:])
            for k in range(17):
                nc.sync.dma_start(w1f[:, k * H1:(k + 1) * H1],
                                  W1D[k * 128:(k + 1) * 128, :])
            for k in range(4):
                nc.sync.dma_start(w2f[:, k * F:(k + 1) * F],
                                  W2D[k * 128:(k + 1) * 128, :])
            nc.sync.dma_start(b1f[:], B1D[:])
            nc.sync.dma_start(b2f[:], B2D[:])
            nc.sync.dma_start(onef[:], ONED[:])

            w1r = wp.tile([128, 17 * H1], dt.float32r)
            w2r = wp.tile([128, 4 * F], dt.float32r)
            b1r = wp.tile([1, H1], dt.float32r)
            b2r = wp.tile([1, F], dt.float32r)
            oner = wp.tile([1, 128], dt.float32r)
            nc.scalar.activation(w1r[:], w1f[:], AF.Copy)
            nc.scalar.activation(w2r[:], w2f[:], AF.Copy)
            nc.scalar.activation(b1r[:], b1f[:], AF.Copy)
            nc.scalar.activation(b2r[:], b2f[:], AF.Copy)
            nc.scalar.activation(oner[:], onef[:], AF.Copy)

            # absorber: first PE touch of ident
            scrapt = psa.tile([1, 128], dt.float32)
            nc.tensor.transpose(scrapt[:], ident[:, :1], ident[:])

            srow = smu[:, 0:1024].rearrange("p (f a) -> p f a", f=16)
            murow = smu[:, 1024:2048].rearrange("p (f a) -> p f a", f=16)

            for i in range(NB):
                hp = pool.tile([128, 132], dt.float32)
                aug = pool.tile([5, 256], dt.float32)
                nc.sync.dma_start(hp[:], HPACK[i * 128:(i + 1) * 128, :])
                nc.sync.dma_start(aug[:], AUGT[i * 5:(i + 1) * 5, :])

                # shared 1-bank psum: d2 (0:64), hT (128:256), out2 (256:384)
                scratch = psum.tile([128, 512], dt.float32)
                nc.tensor.matmul(scratch[0:64, 0:64], aug[0:5, 0:64],
                                 aug[0:5, 128:192], start=True, stop=True)
                nc.tensor.matmul(scratch[64:128, 0:64], aug[0:5, 64:128],
                                 aug[0:5, 192:256], start=True, stop=True)
                d2c = pool.tile([128, 64], dt.float32)
                nc.scalar.activation(d2c[:], scratch[:, 0:64], AF.Relu)
                d_sb = pool.tile([128, 64], dt.float32)
                nc.scalar.activation(d_sb[:], d2c[:], AF.Sqrt)

                # RBF: g[p=(mol,b), f*64+a] = exp(-(d*s_f - mu_f*s_f)^2)
                d_b = d_sb[:].unsqueeze(1).broadcast_to([128, 16, 64])
                t1 = pool.tile([128, 1024], dt.float32)
                nc.vector.tensor_tensor(
                    t1[:].rearrange("p (f a) -> p f a", f=16), d_b, srow, ALU.mult)
                t2 = pool.tile([128, 1024], dt.float32)
                nc.vector.tensor_tensor(
                    t2[:].rearrange("p (f a) -> p f a", f=16),
                    t1[:].rearrange("p (f a) -> p f a", f=16), murow, ALU.subtract)
                t3 = pool.tile([128, 1024], dt.float32)
                nc.vector.tensor_tensor(t3[:], t2[:], t2[:], ALU.mult)
                g = pool.tile([128, 1024], dt.float32r)
                nc.scalar.activation(g[:], t3[:], AF.Exp, scale=-1.0)

                # hm[b, c] = h[b, c] * (5 * mask[b])
                hm = pool.tile([128, 128], dt.float32r)
                nc.scalar.activation(hm[:], hp[:, 0:128], AF.Copy,
                                     scale=hp[:, 128:129])

                # vT chunk 0 = hT (raw h transposed)
                nc.tensor.transpose(scratch[:, 128:256], hp[:, 0:128], ident[:])
                vTp = pool.tile([128, 2240], dt.float32r)
                nc.scalar.activation(vTp[:, 0:128], scratch[:, 128:256], AF.Copy)

                # einsum: uT_m[c, f*64+a] = sum_b hm[b,c] g[b, f*64+a]
                uT0 = psum.tile([128, 1024], dt.float32)
                uT1 = psum.tile([128, 1024], dt.float32)
                nc.tensor.matmul(uT0[:, 0:512], hm[0:64, :], g[0:64, 0:512],
                                 start=True, stop=True)
                nc.tensor.matmul(uT0[:, 512:1024], hm[0:64, :], g[0:64, 512:1024],
                                 start=True, stop=True)
                nc.tensor.matmul(uT1[:, 0:512], hm[64:128, :], g[64:128, 0:512],
                                 start=True, stop=True)
                nc.tensor.matmul(uT1[:, 512:1024], hm[64:128, :],
                                 g[64:128, 512:1024], start=True, stop=True)

                # scatter uT -> vTp[c, (f+1)*128 + 64*mol + a]
                src0 = uT0[:].rearrange("p (f a) -> p f a", f=16)
                dst0 = vTp[:, 128:2176].rearrange("p (f a) -> p f a", f=16)[:, :, 0:64]
                nc.scalar.activation(dst0, src0, AF.Copy)
                src1 = uT1[:].rearrange("p (f a) -> p f a", f=16)
                dst1 = vTp[:, 192:2240].rearrange("p (f a) -> p f a", f=16)[:, :, 0:64]
                nc.vector.tensor_copy(dst1, src1)

                # MLP1: out1[a, o] = sum_j v[a,j] W1[j,o] + b1[o]
                out1 = psum.tile([128, H1], dt.float32)
                for k in range(17):
                    nc.tensor.matmul(out1[:], vTp[:, k * 128:(k + 1) * 128],
                                     w1r[:, k * H1:(k + 1) * H1],
                                     start=(k == 0), stop=False,
                                     skip_group_check=(k > 0))
                nc.tensor.matmul(out1[:], oner[:], b1r[:], start=False, stop=True,
                                 skip_group_check=True)

                s1 = pool.tile([128, H1], dt.float32)
                nc.scalar.activation(s1[:], out1[:], AF.Silu)

                # transpose s1 -> s1T (4 chunks), round to fp32r
                s1T = psum.tile([128, H1], dt.float32)
                for k in range(4):
                    nc.tensor.transpose(s1T[:, k * 128:(k + 1) * 128],
                                        s1[:, k * 128:(k + 1) * 128], ident[:])
                s1Tr = pool.tile([128, H1], dt.float32r)
                nc.scalar.activation(s1Tr[:], s1T[:], AF.Copy)

                # MLP2: out2[a, c] = sum_o s1[a,o] W2[o,c] + b2[c]
                for k in range(4):
                    nc.tensor.matmul(scratch[:, 256:384],
                                     s1Tr[:, k * 128:(k + 1) * 128],
                                     w2r[:, k * F:(k + 1) * F],
                                     start=(k == 0), stop=False,
                                     skip_group_check=(k > 0))
                nc.tensor.matmul(scratch[:, 256:384], oner[:], b2r[:],
                                 start=False, stop=True, skip_group_check=True)

                # h_new = h + out2 * (0.1 * mask[a])
                upd = pool.tile([128, F], dt.float32)
                nc.scalar.activation(upd[:], scratch[:, 256:384], AF.Copy,
                                     scale=hp[:, 129:130])
                hnew = pool.tile([128, F], dt.float32)
                nc.vector.tensor_tensor(hnew[:], hp[:, 0:128], upd[:], ALU.add)
                nc.sync.dma_start(HNEW[i * 128:(i + 1) * 128, :], hnew[:])
    nc.compile()
    return nc


def _get_nc():
    global _NC
    if _NC is None:
        _NC = _build()
    return _NC


def _prep_core(z, r, h, c):
    zl = np.asarray(z[c * MPC:(c + 1) * MPC])
    rl = np.asarray(r[c * MPC:(c + 1) * MPC], dtype=np.float32)
    hl = np.asarray(h[c * MPC:(c + 1) * MPC], dtype=np.float32)
    mask = (zl > -1).astype(np.float32).reshape(NB * 128, 1)
    hp = np.concatenate(
        [hl.reshape(NB * 128, F), WSCALE * mask, UPDATE * mask,
         np.zeros((NB * 128, 2), np.float32)], axis=1).astype(np.float32)
    rb = rl.reshape(NB, 128, 3)
    rt = np.ascontiguousarray(rb.transpose(0, 2, 1))          # [NB,3,128]
    s2 = (rb * rb).sum(-1, dtype=np.float32)[:, None, :]      # [NB,1,128]
    on = np.ones((NB, 1, 128), np.float32)
    lhs = np.concatenate([rt, s2, on], 1)                     # [NB,5,128]
    rhs = np.concatenate([-2.0 * rt, on, s2], 1)
    aug = np.concatenate([lhs, rhs], 2).astype(np.float32).reshape(NB * 5, 256)
    return hp, aug


def _run(inputs, trace=False):
    from concourse.bass_utils import run_bass_kernel_spmd
    z = np.asarray(inputs["z"])
    r = np.asarray(inputs["r"])
    h = np.asarray(inputs["h"], dtype=np.float32)
    distances = np.asarray(inputs["distances"], dtype=np.float32)
    widths = np.asarray(inputs["widths"], dtype=np.float32)
    W1 = np.asarray(inputs["W1"], dtype=np.float32)
    b1 = np.asarray(inputs["b1"], dtype=np.float32)
    W2 = np.asarray(inputs["W2"], dtype=np.float32)
    b2 = np.asarray(inputs["b2"], dtype=np.float32)

    nc = _get_nc()

    s = (1.0 / np.sqrt(widths)).astype(np.float32)
    srow = np.repeat(s, MA)[None, :].repeat(128, 0)
    murow = np.repeat(distances * s, MA)[None, :].repeat(128, 0)
    smu = np.concatenate([srow, murow], 1).astype(np.float32)
    common = {
        "SMU": smu,
        "IDN": np.eye(128, dtype=np.float32),
        "W1D": np.ascontiguousarray(W1),
        "W2D": np.ascontiguousarray(W2),
        "B1D": np.ascontiguousarray(b1[None, :]),
        "B2D": np.ascontiguousarray(b2[None, :]),
        "ONED": np.ones((1, 128), np.float32),
    }
    in_maps = []
    for c in range(NCORES):
        hp, aug = _prep_core(z, r, h, c)
        m = dict(common)
        m["HPACK"] = hp
        m["AUGT"] = aug
        in_maps.append(m)

    res = run_bass_kernel_spmd(nc, in_maps, list(range(NCORES)), trace=trace)
    hn = np.concatenate(
        [res.results[c]["HNEW"].reshape(MPC, MA, F) for c in range(NCORES)], 0)
    return (z, r, hn.astype(np.float32)), res


def kernel(**inputs):
    outs, _ = _run(inputs, trace=False)
    return outs
